# revision 23
# baseline (speedup 1.0000x reference)
"""Multi-head causal attention with RoPE on 8 Trainium2 NeuronCores.

Sharding: tensor-parallel over heads x data-parallel over batch.
Core c handles batch b = c//4 and heads [4*(c%4), 4*(c%4)+4) (Hl=256 of Hd=1024).
Each core computes q/k/v projections for its head slice (column-split Wq/Wk/Wv),
RoPE, causal softmax attention, and a partial output projection (row-split Wo).
The host sums the 4 fp16 partial outputs per batch (the "all-reduce").

Device layout/schedule highlights (per core, S=2048, E=1024, Hl=256, D=64):
  - all matmul operands fp16 (fp32 PSUM); fp16 RoPE tables (2x DVE rate) and
    fp16 partial outputs (half the store traffic).
  - projections stream j-outer so every arriving 512-column slice of x
    immediately yields 4 q/k projection groups + 4 v chunks; RoPE applied
    per 512-column slice right after each eviction, so attention (which
    depends on q/k/v subtiles only) overlaps the projection phase.
  - scores computed transposed (keys on partitions) per 128-key chunk with
    both heads of a slab side by side in one 2-bank PSUM tile: one exp
    ACTIVATE and one affine_select per chunk. The two score matmuls run
    concurrently on disjoint PE row groups (head0 rows 0-63, head1 64-127)
    and MUST write different PSUM banks (same-bank concurrent drains are a
    fatal HW collision - do not "pack" trimmed chunks into one bank).
  - the trailing three diagonal chunks of each block are width-trimmed
    (128/256/384 columns masked off); softmax Z rides a ones-column in v,
    normalization uses reciprocal_approx_fast on a DRAM-broadcast of Z.
  - double-buffered score tiles and pv accumulators (csc=2, cpv=2, 8 PSUM
    banks total) so block transitions don't stall the PE queue.
  - warmup matmuls keep the PE HAM clock-gate warm through the input-DMA
    ramp; inputs stream over the three DMA-capable queues (sync/gpsimd/
    scalar); outputs drain over all three.

Measured: 185.9 us HW exec (best 185.5), rel err 4.9e-4 vs fp32 reference.
"""
import sys

sys.path.insert(0, "/opt/trn_rl_repo")
import numpy as np  # noqa: E402

N_HEADS = 16
B, S, E, HD = 2, 2048, 1024, 1024
D = HD // N_HEADS  # 64
HPC = 4            # heads per core
HL = HPC * D       # 256
NCORES = 8
ROPE_BASE = 10000.0

_built = None


def _build_nc():
    import concourse.bass as bass
    import concourse.tile as tile
    from concourse import bacc, mybir

    F32 = mybir.dt.float32
    F16 = mybir.dt.float16
    Exp = mybir.ActivationFunctionType.Exp
    is_ge = mybir.AluOpType.is_ge
    ts = bass.ts

    nc = bacc.Bacc("TRN2", target_bir_lowering=False, debug=False)
    xT_d = nc.dram_tensor("xT", [E, S], F16, kind="ExternalInput").ap()
    wq_d = nc.dram_tensor("wq", [E, HL], F16, kind="ExternalInput").ap()
    wk_d = nc.dram_tensor("wk", [E, HL], F16, kind="ExternalInput").ap()
    wv_d = nc.dram_tensor("wv", [E, HL], F16, kind="ExternalInput").ap()
    wo_d = nc.dram_tensor("wo", [HL, E], F16, kind="ExternalInput").ap()
    cos_d = nc.dram_tensor("cosx", [128, S], F16, kind="ExternalInput").ap()
    sin_d = nc.dram_tensor("sinx", [128, S], F16, kind="ExternalInput").ap()
    out_d = nc.dram_tensor("out", [S, E], F16, kind="ExternalOutput").ap()
    zscr_d = nc.dram_tensor("zscr", [HPC, S], F32).ap()  # internal scratch

    ECH = E // 128   # 8 e-chunks
    SCH = S // 128   # 16 seq chunks
    SB = S // 512    # 4 seq blocks
    swap_mask = []
    for i in range(16):
        swap_mask += [2 * i + 1, 2 * i]

    with tile.TileContext(nc) as tc:
        with (
            tc.tile_pool(name="persist", bufs=1) as pp,
            tc.tile_pool(name="evict", bufs=6) as ev,
        ):
            qT = [pp.tile([128, S], F16, tag=f"qT{c}", name=f"qT{c}") for c in range(2)]
            kT = [pp.tile([128, S], F16, tag=f"kT{c}", name=f"kT{c}") for c in range(2)]
            vt = [pp.tile([128, HPC * (D + 1)], F16, tag=f"v{t}", name=f"v{t}")
                  for t in range(SCH)]
            oT = [pp.tile([128, S], F16, tag=f"oT{c}", name=f"oT{c}") for c in range(2)]
            cosx = pp.tile([128, S], F16, tag="cosx", name="cosx")
            sinx = pp.tile([128, S], F16, tag="sinx", name="sinx")
            wo_t = pp.tile([128, 2, E], F16, tag="wo", name="wo")

            with (
                tc.tile_pool(name="bx", bufs=1) as bx,
                tc.tile_pool(name="bswp", bufs=2) as bswp,
                tc.tile_pool(name="bps", bufs=7, space="PSUM") as bps,
                tc.tile_pool(name="bwarm", bufs=1, space="PSUM") as bwarm,
            ):
                junk = bx.tile([128, 512], F16, tag="junk", name="junk")
                wps = bwarm.tile([128, 512], F32, tag="warm", name="warm")
                nc.gpsimd.memset(junk[:], 0.0)
                for _ in range(12):
                    nc.tensor.matmul(wps[:], junk[:, 0:128], junk[:],
                                     start=True, stop=True)

                wq_t = bx.tile([128, ECH, HL], F16, tag="wq", name="wq")
                wk_t = bx.tile([128, ECH, HL], F16, tag="wk", name="wk")
                wv_t = bx.tile([128, ECH, HL], F16, tag="wv", name="wv")
                def wdma(eng, w_t_, w_d_):
                    eng.dma_start(
                        out=w_t_[:],
                        in_=w_d_.rearrange("(c p) m -> p c m", p=128),
                    )
                xt = [bx.tile([128, S], F16, tag=f"x{e}", name=f"x{e}")
                      for e in range(ECH)]

                def xdma(eng, e, j):
                    eng.dma_start(
                        out=xt[e][:, ts(j, 512)],
                        in_=xT_d[e * 128:(e + 1) * 128, ts(j, 512)],
                    )
                wdma(nc.sync, wq_t, wq_d)
                wdma(nc.scalar, wk_t, wk_d)
                for j in range(SB):
                    for e in range(0, ECH, 2):
                        xdma(nc.sync, e, j)
                    for e in range(1, ECH, 2):
                        xdma(nc.gpsimd, e, j)
                wdma(nc.scalar, wv_t, wv_d)
                nc.scalar.dma_start(out=cosx[:], in_=cos_d)
                nc.scalar.dma_start(out=sinx[:], in_=sin_d)
                nc.scalar.dma_start(
                    out=wo_t[:],
                    in_=wo_d.rearrange("(c p) e -> p c e", p=128),
                )

                def rope_slice(dest, c, j):
                    sw = bswp.tile([128, 512], F16, tag="swp", name="swp")
                    sl = bass.ts(j, 512)
                    nc.vector.stream_shuffle(
                        out=sw[:], in_=dest[c][:, sl], mask=swap_mask
                    )
                    nc.vector.tensor_mul(out=sw[:], in0=sw[:], in1=sinx[:, sl])
                    nc.vector.tensor_mul(
                        out=dest[c][:, sl], in0=dest[c][:, sl], in1=cosx[:, sl]
                    )
                    nc.vector.tensor_add(
                        out=dest[c][:, sl], in0=dest[c][:, sl], in1=sw[:]
                    )

                nev = 0
                for j in range(SB):
                    for w_t_, dest in ((wk_t, kT), (wq_t, qT)):
                        for m in range(2):
                            ps = bps.tile([128, 512], F32, tag="mm", name="mm")
                            for e in range(ECH):
                                nc.tensor.matmul(
                                    ps[:],
                                    w_t_[:, e, m * 128:(m + 1) * 128],
                                    xt[e][:, ts(j, 512)],
                                    start=(e == 0),
                                    stop=(e == ECH - 1),
                                )
                            if nev % 2 == 0:
                                nc.vector.tensor_copy(
                                    out=dest[m][:, ts(j, 512)], in_=ps[:]
                                )
                            else:
                                nc.scalar.copy(
                                    out=dest[m][:, ts(j, 512)], in_=ps[:]
                                )
                            nev += 1
                            rope_slice(dest, m, j)
                    for t in range(4 * j, 4 * j + 4):
                        nc.gpsimd.memset(
                            vt[t].rearrange("p (h c) -> p h c", c=D + 1)[:, :, D:D + 1],
                            1.0,
                        )
                        ps = bps.tile([128, HL], F32, tag="mm", name="mmv")
                        for e in range(ECH):
                            nc.tensor.matmul(
                                ps[:],
                                xt[e][:, ts(t, 128)],
                                wv_t[:, e, :],
                                start=(e == 0),
                                stop=(e == ECH - 1),
                            )
                        if t % 2 == 0:
                            nc.vector.tensor_copy(
                                out=vt[t].rearrange("p (h c) -> p h c", c=D + 1)[:, :, 0:D],
                                in_=ps.rearrange("p (h c) -> p h c", c=D),
                            )
                        else:
                            nc.scalar.copy(
                                out=vt[t].rearrange("p (h c) -> p h c", c=D + 1)[:, :, 0:D],
                                in_=ps.rearrange("p (h c) -> p h c", c=D),
                            )

            with (
                tc.tile_pool(name="cexp", bufs=6) as cexp,
                tc.tile_pool(name="cz", bufs=4) as cz,
                tc.tile_pool(name="crb", bufs=3) as crb,
                tc.tile_pool(name="csc", bufs=2, space="PSUM") as csc,
                tc.tile_pool(name="cpv", bufs=2, space="PSUM") as cpv,
            ):
                def qksv(c):
                    hs = [2 * c, 2 * c + 1]
                    qs = [qT[c][0:64, :], qT[c][64:128, :]]
                    ks = [kT[c][0:64, :], kT[c][64:128, :]]
                    vs = [
                        [vt[t].rearrange("p (h c) -> p h c", c=D + 1)[:, h, :]
                         for t in range(SCH)]
                        for h in hs
                    ]
                    return hs, qs, ks, vs

                units = []
                for c, j in [(0, 0), (0, 1), (1, 0), (1, 1),
                             (0, 2), (1, 2), (0, 3), (1, 3)]:
                    nt = 4 * (j + 1)
                    for t in range(nt):
                        units.append((c, j, t, nt))
                sc_of = {}
                pv_of = {}

                def trim_off(t, nt):
                    if t == nt - 3:
                        return 128
                    if t == nt - 2:
                        return 256
                    if t == nt - 1:
                        return 384
                    return 0

                def emit_sc(u):
                    c, j, t, nt = u
                    _, qs, ks, _ = qksv(c)
                    off = trim_off(t, nt)
                    sc = csc.tile([128, 1024], F32, tag="sc", name="sc")
                    for i in range(2):
                        nc.tensor.matmul(
                            sc[:, i * 512 + off:(i + 1) * 512],
                            ks[i][:, ts(t, 128)],
                            qs[i][:, j * 512 + off:(j + 1) * 512],
                            start=True,
                            stop=True,
                        )
                    sc_of[u] = sc

                def emit_consume(u):
                    c, j, t, nt = u
                    hs, _, _, vs = qksv(c)
                    if t == 0:
                        pv_of[(c, j)] = [
                            cpv.tile([65, 512], F32, tag=f"pv{i}", name=f"pv{i}")
                            for i in range(2)
                        ]
                    pv = pv_of[(c, j)]
                    sc = sc_of.pop(u)
                    off = trim_off(t, nt)
                    exm = cexp.tile([128, 1024], F16, tag="ex", name="ex")
                    if off:
                        nc.scalar.activation(
                            out=exm.rearrange("p (h q) -> p h q", h=2)[:, :, off:512],
                            in_=sc.rearrange("p (h q) -> p h q", h=2)[:, :, off:512],
                            func=Exp, scale=0.125,
                        )
                    else:
                        nc.scalar.activation(
                            out=exm[:], in_=sc[:], func=Exp, scale=0.125
                        )
                    if t >= nt - 4:
                        ex3 = exm.rearrange("p (h q) -> p h q", h=2)[:, :, off:512]
                        nc.gpsimd.affine_select(
                            out=ex3, in_=ex3,
                            compare_op=is_ge,
                            fill=0.0,
                            base=(j * 512 - t * 128) + off,
                            channel_multiplier=-1,
                            pattern=[[0, 2], [1, 512 - off]],
                        )
                    for i in range(2):
                        nc.tensor.matmul(
                            pv[i][:, off:512],
                            vs[i][t],
                            exm[:, i * 512 + off:(i + 1) * 512],
                            start=(t == 0),
                            stop=(t == nt - 1),
                        )
                    if t == nt - 1:
                        zq = cz.tile([65, 2, 512], F32, tag="zq", name="zq")
                        for i in range(2):
                            nc.vector.tensor_copy(
                                out=oT[c][i * 64:(i + 1) * 64, ts(j, 512)],
                                in_=pv[i][0:64, :],
                            )
                            nc.vector.tensor_copy(
                                out=zq[64:65, i, :], in_=pv[i][64:65, :]
                            )
                        for i in range(2):
                            nc.sync.dma_start(
                                out=zscr_d[hs[i], ts(j, 512)],
                                in_=zq[64:65, i, :],
                            )
                        rb = crb.tile([128, 512], F32, tag="rb", name="rb")
                        for i in range(2):
                            nc.sync.dma_start(
                                out=rb[i * 64:(i + 1) * 64, :],
                                in_=zscr_d[hs[i]:hs[i] + 1, ts(j, 512)]
                                .to_broadcast((64, 512)),
                            )
                        rbr = crb.tile([128, 512], F32, tag="rbr", name="rbr")
                        nc.vector.reciprocal_approx_fast(out=rbr[:], in_=rb[:])
                        nc.vector.tensor_mul(
                            out=oT[c][:, ts(j, 512)],
                            in0=oT[c][:, ts(j, 512)],
                            in1=rbr[:],
                        )

                emit_sc(units[0])
                for un in range(1, len(units)):
                    emit_sc(units[un])
                    emit_consume(units[un - 1])
                emit_consume(units[-1])

            with tc.tile_pool(name="dps", bufs=8, space="PSUM") as dps:
                oq = [nc.sync, nc.gpsimd, nc.scalar]
                for t in range(SCH):
                    for n in range(2):
                        ps = dps.tile([128, 512], F32, tag="wo", name="wops")
                        for c in range(2):
                            nc.tensor.matmul(
                                ps[:],
                                oT[c][:, ts(t, 128)],
                                wo_t[:, c, ts(n, 512)],
                                start=(c == 0),
                                stop=(c == 1),
                            )
                        ot = ev.tile([128, 512], F16, tag="out", name="oev")
                        if (2 * t + n) % 2 == 0:
                            nc.vector.tensor_copy(out=ot[:], in_=ps[:])
                        else:
                            nc.scalar.copy(out=ot[:], in_=ps[:])
                        oq[(2 * t + n) % 3].dma_start(
                            out=out_d[ts(t, 128), ts(n, 512)], in_=ot[:]
                        )

    nc.compile()
    return nc


def _rope_tables():
    iexp = np.arange(0, D, 2, dtype=np.float32) / np.float32(D)
    inv_freq = np.reciprocal(np.power(np.float32(ROPE_BASE), iexp))
    ang = np.arange(S, dtype=np.float32)[:, None] * inv_freq[None, :]
    cos = np.cos(ang).astype(np.float32)
    sin = np.sin(ang).astype(np.float32)
    cosx = np.empty((64, S), dtype=np.float32)
    sinx = np.empty((64, S), dtype=np.float32)
    cosx[0::2] = cos.T
    cosx[1::2] = cos.T
    sinx[0::2] = -sin.T
    sinx[1::2] = sin.T
    return (np.tile(cosx, (2, 1)).astype(np.float16),
            np.tile(sinx, (2, 1)).astype(np.float16))


def get_nc():
    global _built
    if _built is None:
        _built = _build_nc()
    return _built


def make_in_maps(x, Wq, Wk, Wv, Wo):
    cosx, sinx = _rope_tables()
    in_maps = []
    for c in range(NCORES):
        b, g = c // 4, c % 4
        sl = slice(g * HL, (g + 1) * HL)
        in_maps.append({
            "xT": np.ascontiguousarray(x[b].T).astype(np.float16),
            "wq": np.ascontiguousarray(Wq[:, sl]).astype(np.float16),
            "wk": np.ascontiguousarray(Wk[:, sl]).astype(np.float16),
            "wv": np.ascontiguousarray(Wv[:, sl]).astype(np.float16),
            "wo": np.ascontiguousarray(Wo[sl, :]).astype(np.float16),
            "cosx": cosx,
            "sinx": sinx,
        })
    return in_maps


def gather(results):
    out = np.empty((B, S, E), dtype=np.float32)
    for b in range(B):
        acc = results[4 * b]["out"].astype(np.float32)
        for g in range(1, 4):
            acc += results[4 * b + g]["out"].astype(np.float32)
        out[b] = acc
    return out


def kernel(x, Wq, Wk, Wv, Wo):
    from concourse.bass_utils import run_bass_kernel_spmd

    nc = get_nc()
    in_maps = make_in_maps(
        np.asarray(x), np.asarray(Wq), np.asarray(Wk), np.asarray(Wv), np.asarray(Wo)
    )
    res = run_bass_kernel_spmd(nc, in_maps, list(range(NCORES)))
    return gather(res.results)
